# revision 3
# baseline (speedup 1.0000x reference)
"""RGBD channel-attention (CAM) module on 8 Trainium2 NeuronCores.

Per batch b (one per core, pure data-parallel):
    q  = x_rgb[b].reshape(C, N)          C=512, N=4096
    qd = x_dep[b].reshape(C, N)
    S  = q @ q.T + qd @ qd.T             (512 x 512, symmetric)
    att = softmax(-S, axis=-1)           (max-shift trick cancels in softmax)
    out = gamma * (att @ q) + x_rgb[b]

On-chip schedule per core:
  phase E: build qT / qdT tiles via PE transposes (128x128 via identity
           matmul), accumulate S = sum_k qT_k^T @ qT_k over both inputs
           into 4 PSUM banks (one per 128-row tile of S).
  softmax: rowwise m=min(S), unnormalized att = exp(-S + m) via ACT with
           accumulated row sum; normalization deferred to the output scale.
  attT:    16 PE transposes (att is needed d-major as matmul lhsT).
  out:     att^T.T @ q accumulated over d-tiles; PSUM scaled by
           gamma/rowsum (per-partition) on ACT, residual-added to x on DVE,
           DMA'd out.
"""

import sys

if "/opt/trn_rl_repo" not in sys.path:
    sys.path.insert(0, "/opt/trn_rl_repo")

import numpy as np

import concourse.bacc as bacc
import concourse.mybir as mybir
import concourse.tile as tile
from concourse import bass_utils
from concourse.masks import make_identity

P = 128          # partitions
C = 512          # channels
N = 4096         # H*W
CT = C // P      # 4 channel tiles
KT = N // P      # 32 contraction tiles per input
FREE = 512       # matmul moving free dim (fp32 max)
NT = N // FREE   # 8 output column tiles
F32 = mybir.dt.float32

_NC_CACHE = None


def _emit(ctx, tc, nc, x, xd, g, o):
    xa = x.ap().rearrange("(t p) n -> t p n", p=P)    # [CT, P, N]
    xda = xd.ap().rearrange("(t p) n -> t p n", p=P)
    oa = o.ap().rearrange("(t p) n -> t p n", p=P)

    const = ctx.enter_context(tc.tile_pool(name="const", bufs=1))
    qpool = ctx.enter_context(tc.tile_pool(name="qpool", bufs=1))
    stream = ctx.enter_context(tc.tile_pool(name="stream", bufs=3))
    small = ctx.enter_context(tc.tile_pool(name="small", bufs=1))
    ostream = ctx.enter_context(tc.tile_pool(name="ostream", bufs=3))
    spsum = ctx.enter_context(tc.tile_pool(name="spsum", bufs=1, space="PSUM"))
    tpsum = ctx.enter_context(tc.tile_pool(name="tpsum", bufs=2, space="PSUM"))
    opsum = ctx.enter_context(tc.tile_pool(name="opsum", bufs=2, space="PSUM"))

    identity = const.tile([P, P], F32)
    make_identity(nc, identity)
    gamma_sb = const.tile([P, 1], F32)
    nc.sync.dma_start(gamma_sb[:], g.ap())

    # resident natural-layout inputs (q doubles as the residual x)
    q_nat = qpool.tile([P, CT, N], F32, name="q_nat", tag="q_nat")
    qd_nat = qpool.tile([P, CT, N], F32, name="qd_nat", tag="qd_nat")
    for t in range(CT):
        for h in range(2):
            sl = slice(h * 2048, (h + 1) * 2048)
            nc.sync.dma_start(q_nat[:, t, sl], xa[t, :, sl])
    for t in range(CT):
        for h in range(2):
            sl = slice(h * 2048, (h + 1) * 2048)
            nc.sync.dma_start(qd_nat[:, t, sl], xda[t, :, sl])

    # S accumulators: 4 PSUM banks, one per 128-row tile of S
    s_ps = [spsum.tile([P, FREE], F32, name=f"s{m}", tag=f"s{m}") for m in range(CT)]

    # --- phase E: S = q@qT + qd@qdT, contraction streamed k tile by k tile
    steps = [(q_nat, k) for k in range(KT)] + [(qd_nat, k) for k in range(KT)]
    n_steps = len(steps)
    pend = {}

    def emit_transpose(i):
        src, k = steps[i]
        tp = tpsum.tile([P, FREE], F32, tag="tp")
        for t in range(CT):
            nc.tensor.transpose(
                tp[:, t * P : (t + 1) * P], src[:, t, k * P : (k + 1) * P], identity
            )
        qt = stream.tile([P, FREE], F32, tag="qt")
        nc.vector.tensor_copy(out=qt[:], in_=tp[:])
        pend[i] = qt

    def emit_matmuls(i):
        qt = pend.pop(i)
        for m in range(CT):
            nc.tensor.matmul(
                s_ps[m][:],
                qt[:, m * P : (m + 1) * P],
                qt[:],
                start=(i == 0),
                stop=(i == n_steps - 1),
            )

    # skew by one step so PE transposes of step i+1 overlap the DVE copy of i
    emit_transpose(0)
    for i in range(1, n_steps):
        emit_transpose(i)
        emit_matmuls(i - 1)
    emit_matmuls(n_steps - 1)

    # --- softmax over rows of -S (stabilized by row max of -S == -min S)
    att = qpool.tile([P, CT, FREE], F32, name="att", tag="att")
    rg = []
    for m in range(CT):
        mn = small.tile([P, 1], F32, tag=f"mn{m}")
        nc.vector.tensor_reduce(
            out=mn[:], in_=s_ps[m][:], axis=mybir.AxisListType.X, op=mybir.AluOpType.min
        )
        z = small.tile([P, 1], F32, tag=f"z{m}")
        nc.scalar.activation(
            att[:, m, :],
            s_ps[m][:],
            mybir.ActivationFunctionType.Exp,
            bias=mn[:],
            scale=-1.0,
            accum_out=z[:],
        )
        r = small.tile([P, 1], F32, tag=f"r{m}")
        nc.vector.reciprocal(r[:], z[:])
        rgm = small.tile([P, 1], F32, tag=f"rg{m}")
        nc.vector.tensor_mul(out=rgm[:], in0=r[:], in1=gamma_sb[:])
        rg.append(rgm)

    # --- attT (d-major view of unnormalized att) via 16 PE transposes
    attT = qpool.tile([P, CT, FREE], F32, name="attT", tag="attT")
    for mj in range(CT):
        tp = tpsum.tile([P, FREE], F32, tag="tp")
        for mi in range(CT):
            nc.tensor.transpose(
                tp[:, mi * P : (mi + 1) * P],
                att[:, mi, mj * P : (mj + 1) * P],
                identity,
            )
        nc.vector.tensor_copy(out=attT[:, mj, :], in_=tp[:])

    # --- out = rg * (attT.T @ q) + x, streamed over 8 column tiles
    for nt in range(NT):
        nsl = slice(nt * FREE, (nt + 1) * FREE)
        for m in range(CT):
            op_ = opsum.tile([P, FREE], F32, tag="op")
            for kk in range(CT):
                nc.tensor.matmul(
                    op_[:],
                    attT[:, kk, m * P : (m + 1) * P],
                    q_nat[:, kk, nsl],
                    start=(kk == 0),
                    stop=(kk == CT - 1),
                )
            t_sb = ostream.tile([P, FREE], F32, tag="t")
            nc.scalar.mul(t_sb[:], op_[:], rg[m][:])
            o_sb = ostream.tile([P, FREE], F32, tag="o")
            nc.vector.tensor_add(out=o_sb[:], in0=t_sb[:], in1=q_nat[:, m, nsl])
            nc.sync.dma_start(oa[m, :, nsl], o_sb[:])


def _build_program():
    global _NC_CACHE
    if _NC_CACHE is not None:
        return _NC_CACHE
    nc = bacc.Bacc("TRN2", target_bir_lowering=False, debug=False)
    x = nc.dram_tensor("x", [C, N], F32, kind="ExternalInput")
    xd = nc.dram_tensor("xd", [C, N], F32, kind="ExternalInput")
    g = nc.dram_tensor("g", [P, 1], F32, kind="ExternalInput")
    o = nc.dram_tensor("o", [C, N], F32, kind="ExternalOutput")
    from contextlib import ExitStack

    with tile.TileContext(nc) as tc, ExitStack() as ctx:
        _emit(ctx, tc, nc, x, xd, g, o)
    nc.compile()
    _NC_CACHE = nc
    return nc


def kernel(x_rgb: np.ndarray, x_dep: np.ndarray, gamma: np.ndarray) -> np.ndarray:
    B, Cc, H, W = x_rgb.shape
    assert (B, Cc, H * W) == (8, C, N), (B, Cc, H, W)
    nc = _build_program()
    g128 = np.ascontiguousarray(
        np.broadcast_to(np.float32(gamma).reshape(1, 1), (P, 1)), dtype=np.float32
    )
    in_maps = [
        {
            "x": np.ascontiguousarray(x_rgb[b].reshape(C, N), dtype=np.float32),
            "xd": np.ascontiguousarray(x_dep[b].reshape(C, N), dtype=np.float32),
            "g": g128,
        }
        for b in range(B)
    ]
    res = bass_utils.run_bass_kernel_spmd(nc, in_maps, core_ids=list(range(B)))
    out = np.stack([res.results[b]["o"].reshape(Cc, H, W) for b in range(B)])
    return out.astype(np.float32)


# revision 10
# speedup vs baseline: 2.4311x; 2.4311x over previous
"""RGBD channel-attention (CAM) module on 8 Trainium2 NeuronCores.

Per batch b (one per core, pure data-parallel):
    q  = x_rgb[b].reshape(C, N)          C=512, N=4096
    qd = x_dep[b].reshape(C, N)
    S  = q @ q.T + qd @ qd.T             (512 x 512, symmetric)
    att = softmax(-S, axis=-1)           (max-shift trick cancels in softmax)
    out = gamma * (att @ q) + x_rgb[b]

On-chip schedule per core:
  phase E: build qT / qdT tiles via PE transposes (128x128 via identity
           matmul), accumulate S = sum_k qT_k^T @ qT_k over both inputs
           into 4 PSUM banks (one per 128-row tile of S).
  softmax: rowwise m=min(S), unnormalized att = exp(-S + m) via ACT with
           accumulated row sum; normalization deferred to the output scale.
  attT:    16 PE transposes (att is needed d-major as matmul lhsT).
  out:     att^T.T @ q accumulated over d-tiles; PSUM scaled by
           gamma/rowsum (per-partition) on ACT, residual-added to x on DVE,
           DMA'd out.
"""

import sys

if "/opt/trn_rl_repo" not in sys.path:
    sys.path.insert(0, "/opt/trn_rl_repo")

import numpy as np

import concourse.bacc as bacc
import concourse.mybir as mybir
import concourse.tile as tile
from concourse import bass_utils
from concourse.masks import make_identity

P = 128          # partitions
C = 512          # channels
N = 4096         # H*W
CT = C // P      # 4 channel tiles
KT = N // P      # 32 contraction tiles per input
FREE = 512       # matmul moving free dim (fp32 max)
NT = N // FREE   # 8 output column tiles
F32 = mybir.dt.float32
F32R = mybir.dt.float32r  # same bits as f32; PE runs full-rate (vs 4 cyc/row for f32)

_NC_CACHE = None


def _emit(ctx, tc, nc, x, xd, g, o):
    xa = x.ap().rearrange("(t p) n -> t p n", p=P)    # [CT, P, N]
    xda = xd.ap().rearrange("(t p) n -> t p n", p=P)
    oa = o.ap().rearrange("(t p) n -> t p n", p=P)

    const = ctx.enter_context(tc.tile_pool(name="const", bufs=1))
    qpool = ctx.enter_context(tc.tile_pool(name="qpool", bufs=1))
    stream = ctx.enter_context(tc.tile_pool(name="stream", bufs=3))
    small = ctx.enter_context(tc.tile_pool(name="small", bufs=1))
    ostream = ctx.enter_context(tc.tile_pool(name="ostream", bufs=3))
    spsum = ctx.enter_context(tc.tile_pool(name="spsum", bufs=1, space="PSUM"))
    tpsum = ctx.enter_context(tc.tile_pool(name="tpsum", bufs=2, space="PSUM"))
    opsum = ctx.enter_context(tc.tile_pool(name="opsum", bufs=2, space="PSUM"))

    identity = const.tile([P, P], F32)
    make_identity(nc, identity)
    identity_r = const.tile([P, P], F32R, name="identity_r")
    nc.vector.tensor_copy(out=identity_r[:], in_=identity[:])
    gamma_sb = const.tile([P, 1], F32)
    nc.sync.dma_start(gamma_sb[:], g.ap())

    # resident natural-layout inputs (q doubles as the residual x)
    q_nat = qpool.tile([P, CT, N], F32R, name="q_nat", tag="q_nat")
    qd_nat = qpool.tile([P, CT, N], F32R, name="qd_nat", tag="qd_nat")
    for t in range(CT):
        for h in range(2):
            sl = slice(h * 2048, (h + 1) * 2048)
            nc.sync.dma_start(q_nat[:, t, sl], xa.bitcast(F32R)[t, :, sl])
    for t in range(CT):
        for h in range(2):
            sl = slice(h * 2048, (h + 1) * 2048)
            nc.sync.dma_start(qd_nat[:, t, sl], xda.bitcast(F32R)[t, :, sl])

    # S accumulators: 4 PSUM banks, one per 128-row tile of S
    s_ps = [spsum.tile([P, FREE], F32, name=f"s{m}", tag=f"s{m}") for m in range(CT)]

    # --- phase E: S = q@qT + qd@qdT, contraction streamed k tile by k tile
    steps = [(q_nat, k) for k in range(KT)] + [(qd_nat, k) for k in range(KT)]
    n_steps = len(steps)
    pend = {}

    def emit_transpose(i):
        src, k = steps[i]
        tp = tpsum.tile([P, FREE], F32R, tag="tp")
        for t in range(CT):
            nc.tensor.transpose(
                tp[:, t * P : (t + 1) * P],
                src[:, t, k * P : (k + 1) * P],
                identity_r,
            )
        qt = stream.tile([P, FREE], F32R, tag="qt")
        nc.vector.tensor_copy(out=qt[:], in_=tp[:])
        pend[i] = qt

    def emit_matmuls(i):
        qt = pend.pop(i)
        for m in range(CT):
            nc.tensor.matmul(
                s_ps[m][:],
                qt[:, m * P : (m + 1) * P],
                qt[:],
                start=(i == 0),
                stop=(i == n_steps - 1),
            )

    # skew by one step so PE transposes of step i+1 overlap the DVE copy of i
    emit_transpose(0)
    for i in range(1, n_steps):
        emit_transpose(i)
        emit_matmuls(i - 1)
    emit_matmuls(n_steps - 1)

    # --- softmax over rows of -S (stabilized by row max of -S == -min S)
    att = qpool.tile([P, CT, FREE], F32, name="att", tag="att")
    rg = []
    for m in range(CT):
        mn = small.tile([P, 1], F32, tag=f"mn{m}")
        nc.vector.tensor_reduce(
            out=mn[:], in_=s_ps[m][:], axis=mybir.AxisListType.X, op=mybir.AluOpType.min
        )
        z = small.tile([P, 1], F32, tag=f"z{m}")
        nc.scalar.activation(
            att[:, m, :],
            s_ps[m][:],
            mybir.ActivationFunctionType.Exp,
            bias=mn[:],
            scale=-1.0,
            accum_out=z[:],
        )
        r = small.tile([P, 1], F32, tag=f"r{m}")
        nc.vector.reciprocal(r[:], z[:])
        rgm = small.tile([P, 1], F32, tag=f"rg{m}")
        nc.vector.tensor_mul(out=rgm[:], in0=r[:], in1=gamma_sb[:])
        rg.append(rgm)

    # --- attT (d-major view of unnormalized att) via 16 PE transposes
    attT = qpool.tile([P, CT, FREE], F32R, name="attT", tag="attT")
    for mj in range(CT):
        tp = tpsum.tile([P, FREE], F32, tag="tp")
        for mi in range(CT):
            nc.tensor.transpose(
                tp[:, mi * P : (mi + 1) * P],
                att[:, mi, mj * P : (mj + 1) * P],
                identity,
            )
        nc.vector.tensor_copy(out=attT[:, mj, :], in_=tp[:])

    # --- out = rg * (attT.T @ q) + x, streamed over 8 column tiles
    for nt in range(NT):
        nsl = slice(nt * FREE, (nt + 1) * FREE)
        for m in range(CT):
            op_ = opsum.tile([P, FREE], F32, tag="op")
            for kk in range(CT):
                nc.tensor.matmul(
                    op_[:],
                    attT[:, kk, m * P : (m + 1) * P],
                    q_nat[:, kk, nsl],
                    start=(kk == 0),
                    stop=(kk == CT - 1),
                )
            t_sb = ostream.tile([P, FREE], F32, tag="t")
            nc.scalar.mul(t_sb[:], op_[:], rg[m][:])
            o_sb = ostream.tile([P, FREE], F32, tag="o")
            nc.vector.tensor_add(out=o_sb[:], in0=t_sb[:], in1=q_nat.bitcast(F32)[:, m, nsl])
            nc.sync.dma_start(oa[m, :, nsl], o_sb[:])


def _build_program():
    global _NC_CACHE
    if _NC_CACHE is not None:
        return _NC_CACHE
    nc = bacc.Bacc("TRN2", target_bir_lowering=False, debug=False)
    x = nc.dram_tensor("x", [C, N], F32, kind="ExternalInput")
    xd = nc.dram_tensor("xd", [C, N], F32, kind="ExternalInput")
    g = nc.dram_tensor("g", [P, 1], F32, kind="ExternalInput")
    o = nc.dram_tensor("o", [C, N], F32, kind="ExternalOutput")
    from contextlib import ExitStack

    with tile.TileContext(nc) as tc, ExitStack() as ctx:
        _emit(ctx, tc, nc, x, xd, g, o)
    nc.compile()
    _NC_CACHE = nc
    return nc


def kernel(x_rgb: np.ndarray, x_dep: np.ndarray, gamma: np.ndarray) -> np.ndarray:
    B, Cc, H, W = x_rgb.shape
    assert (B, Cc, H * W) == (8, C, N), (B, Cc, H, W)
    nc = _build_program()
    g128 = np.ascontiguousarray(
        np.broadcast_to(np.float32(gamma).reshape(1, 1), (P, 1)), dtype=np.float32
    )
    in_maps = [
        {
            "x": np.ascontiguousarray(x_rgb[b].reshape(C, N), dtype=np.float32),
            "xd": np.ascontiguousarray(x_dep[b].reshape(C, N), dtype=np.float32),
            "g": g128,
        }
        for b in range(B)
    ]
    res = bass_utils.run_bass_kernel_spmd(nc, in_maps, core_ids=list(range(B)))
    out = np.stack([res.results[b]["o"].reshape(Cc, H, W) for b in range(B)])
    return out.astype(np.float32)
